# revision 8
# baseline (speedup 1.0000x reference)
"""Trainium2 Bass kernel for PointsProposalGenerator (conv+heads+boxes+NMS).

Sharding: 8 cores = 2 images x 4 row-strips of 64 conv rows each.
Each core: fp32 3x3 conv (9 shifted matmuls, PSUM accumulate) + ReLU,
fused 19-channel head matmul (1 logit + 18 offsets) in pixel-major form,
min/max over the 9 points BEFORE exp (exact: monotone ops commute with
min/max), box assembly, dense (16384,5) output. Host does the NMS topk:
candidates = top-2048 by device score, scores refined in float64 numpy
(fp32 reference ordering empirically equals f64 ordering), then greedy
IoU suppression (empirically a no-op at 0.7; exact fallback kept).
"""

import numpy as np

NIMG = 2
C = 128
H = W = 256
SR = 64                    # conv rows per core
PADW = W + 2               # 258
INROWS = SR + 2            # 66
XSIZE = INROWS * PADW      # 17028
NPIX = SR * W              # 16384
NG = NPIX // 128           # 128 pixel groups of 128
GH = 19                    # head channels: 1 logit + 9 x-offsets + 9 y-offsets
SCALE = 8.0                # image_w / W
IMGD = 2047.0
NEG = -1.0e30
CH = 512                   # conv chunk (PSUM bank)
GB = 26                    # head groups per PSUM bank

_PROGRAM = None
LAST_RESULT = None
TRACE = False


def _build_program():
    from concourse import bacc
    import concourse.mybir as mybir
    from concourse.tile import TileContext

    f32 = mybir.dt.float32
    nc = bacc.Bacc(None, target_bir_lowering=False)
    XS = nc.declare_dram_parameter("XS", [C, XSIZE], f32, isOutput=False)
    WT = nc.declare_dram_parameter("WT", [C, 9 * C], f32, isOutput=False)
    WH = nc.declare_dram_parameter("WH", [C, GH], f32, isOutput=False)
    GRID = nc.declare_dram_parameter("GRID", [C, 2 * NG], f32, isOutput=False)
    OUT = nc.declare_dram_parameter("OUT", [NPIX, 5], f32, isOutput=True)

    with TileContext(nc) as tc:
        with (
            tc.tile_pool(name="sb", bufs=1) as sb,
            tc.tile_pool(name="wk", bufs=2) as wk,
            tc.tile_pool(name="cps", bufs=2, space="PSUM") as cps,
            tc.tile_pool(name="hps", bufs=2, space="PSUM") as hps,
        ):
            xs_sb = sb.tile([C, XSIZE], f32)
            nc.sync.dma_start(out=xs_sb[:, :], in_=XS[:, :])
            wt_sb = sb.tile([C, 9 * C], f32)
            nc.sync.dma_start(out=wt_sb[:, :], in_=WT[:, :])
            wh_sb = sb.tile([C, GH], f32)
            nc.sync.dma_start(out=wh_sb[:, :], in_=WH[:, :])
            grid_sb = sb.tile([C, 2 * NG], f32)
            nc.sync.dma_start(out=grid_sb[:, :], in_=GRID[:, :])

            t_strip = sb.tile([C, XSIZE], f32)

            # ---- conv: 9 shifted matmuls per chunk over the padded grid ----
            P0 = PADW + 1                       # first valid output position
            VALID = SR * PADW - 2               # 16510 positions cover rows 1..64
            nchunks = (VALID + CH - 1) // CH    # 33
            for k in range(nchunks):
                p0 = P0 + CH * k
                L = min(CH, VALID - CH * k)
                ps = cps.tile([C, CH], f32, tag="conv")
                for t in range(9):
                    dr, dc = t // 3 - 1, t % 3 - 1
                    off = p0 + dr * PADW + dc
                    nc.tensor.matmul(
                        ps[:, 0:L],
                        wt_sb[:, t * C:(t + 1) * C],
                        xs_sb[:, off:off + L],
                        start=(t == 0),
                        stop=(t == 8),
                    )
                nc.scalar.activation(
                    t_strip[:, p0:p0 + L], ps[:, 0:L],
                    mybir.ActivationFunctionType.Relu,
                )

            # ---- head matmuls (pixel-major) + channel min/max extraction ----
            score_all = sb.tile([C, NG], f32)
            minx = sb.tile([C, NG], f32)
            maxx = sb.tile([C, NG], f32)
            miny = sb.tile([C, NG], f32)
            maxy = sb.tile([C, NG], f32)
            for b in range((NG + GB - 1) // GB):
                g0 = b * GB
                gb = min(GB, NG - g0)
                hp = hps.tile([C, GB * GH], f32, tag="head")
                for j in range(gb):
                    g = g0 + j
                    r, hf = g // 2, g % 2
                    toff = (r + 1) * PADW + 1 + hf * 128
                    nc.tensor.matmul(
                        hp[:, j * GH:(j + 1) * GH],
                        t_strip[:, toff:toff + C],
                        wh_sb[:, 0:GH],
                        start=True,
                        stop=True,
                    )
                hs = wk.tile([C, GB * GH], f32, tag="hs")
                nc.scalar.copy(hs[:, 0:gb * GH], hp[:, 0:gb * GH])
                v = hs[:, 0:gb * GH].rearrange("p (g c) -> p g c", c=GH)
                nc.vector.tensor_copy(
                    score_all[:, g0:g0 + gb].unsqueeze(2), v[:, :, 0:1]
                )
                import concourse.mybir as _m
                for dst, c0, op in (
                    (minx, 1, _m.AluOpType.min),
                    (maxx, 1, _m.AluOpType.max),
                    (miny, 10, _m.AluOpType.min),
                    (maxy, 10, _m.AluOpType.max),
                ):
                    w = v[:, :, c0:c0 + 9]
                    t4 = wk.tile([C, GB * 4], f32, tag="t4")
                    v4 = t4[:, 0:gb * 4].rearrange("p (g c) -> p g c", c=4)
                    nc.vector.tensor_tensor(v4, w[:, :, 0:4], w[:, :, 4:8], op)
                    t2 = wk.tile([C, GB * 2], f32, tag="t2")
                    v2 = t2[:, 0:gb * 2].rearrange("p (g c) -> p g c", c=2)
                    nc.vector.tensor_tensor(v2, v4[:, :, 0:2], v4[:, :, 2:4], op)
                    t1 = wk.tile([C, GB], f32, tag="t1")
                    v1 = t1[:, 0:gb].unsqueeze(2)
                    nc.vector.tensor_tensor(v1, v2[:, :, 0:1], v2[:, :, 1:2], op)
                    nc.vector.tensor_tensor(
                        dst[:, g0:g0 + gb].unsqueeze(2), v1, w[:, :, 8:9], op
                    )

            # ---- box math: clip(exp(8*v) + (grid - 1), 0, 2047) ----
            import concourse.mybir as m
            for buf, gcol in ((minx, 0), (maxx, 0), (miny, 1), (maxy, 1)):
                nc.scalar.activation(
                    buf[:, :], buf[:, :], m.ActivationFunctionType.Exp,
                    scale=SCALE,
                )
                nc.vector.tensor_tensor(
                    buf[:, :], buf[:, :],
                    grid_sb[:, gcol * NG:(gcol + 1) * NG], m.AluOpType.add,
                )
                nc.vector.tensor_scalar(
                    buf[:, :], buf[:, :], IMGD, 0.0,
                    op0=m.AluOpType.min, op1=m.AluOpType.max,
                )

            # ---- assemble dense out tile [128, NG*5] and DMA ----
            outt = sb.tile([C, 5 * NG], f32)
            ov = outt[:, :].rearrange("p (g c) -> p g c", c=5)
            for col, src in (
                (0, score_all), (1, minx), (2, miny), (3, maxx), (4, maxy)
            ):
                nc.vector.tensor_copy(
                    ov[:, :, col:col + 1], src[:, :].unsqueeze(2)
                )
            nc.sync.dma_start(
                out=OUT[:, :].rearrange("(r h p) c -> p r h c", r=SR, h=2),
                in_=outt[:, :].rearrange("p (r h c) -> p r h c", r=SR, h=2),
            )

    nc.compile()
    return nc


def _program():
    global _PROGRAM
    if _PROGRAM is None:
        _PROGRAM = _build_program()
    return _PROGRAM


def _greedy_nms_fallback(sc_sorted, boxes_sorted):
    n = len(sc_sorted)
    x1, y1, x2, y2 = (boxes_sorted[:, i] for i in range(4))
    area = (x2 - x1) * (y2 - y1)
    iw = np.clip(np.minimum(x2[:, None], x2[None, :]) -
                 np.maximum(x1[:, None], x1[None, :]), 0.0, None)
    ih = np.clip(np.minimum(y2[:, None], y2[None, :]) -
                 np.maximum(y1[:, None], y1[None, :]), 0.0, None)
    inter = iw * ih
    iou = inter / (area[:, None] + area[None, :] - inter + 1e-9)
    thr = iou > 0.7
    if not np.triu(thr, 1).any():
        return np.zeros(n, bool)
    sup = np.zeros(n, bool)
    for i in range(n):
        if not sup[i]:
            sup |= thr[i] & (np.arange(n) > i)
            sup[i] = False
    return sup


def _prepare_in_maps(inputs):
    x = np.ascontiguousarray(np.asarray(inputs["x"], dtype=np.float32))
    W_conv = np.asarray(inputs["W_conv"], dtype=np.float32)
    W_off = np.asarray(inputs["W_off"], dtype=np.float32)
    W_log = np.asarray(inputs["W_log"], dtype=np.float32)

    WTnp = np.empty((C, 9 * C), np.float32)
    for t in range(9):
        WTnp[:, t * C:(t + 1) * C] = W_conv[:, :, t // 3, t % 3].T
    WHnp = np.empty((C, GH), np.float32)
    WHnp[:, 0] = W_log[0]
    WHnp[:, 1:10] = W_off[0::2].T
    WHnp[:, 10:19] = W_off[1::2].T
    lin = np.linspace(0.0, IMGD, W).astype(np.float32)

    in_maps = []
    for core in range(8):
        img, s = core // 4, core % 4
        r0 = s * SR
        xs_np = np.zeros((C, INROWS, PADW), np.float32)
        lo, hi = max(r0 - 1, 0), min(r0 + SR + 1, H)
        xs_np[:, lo - (r0 - 1):hi - (r0 - 1), 1:W + 1] = x[img, :, lo:hi, :]
        gridnp = np.empty((C, 2 * NG), np.float32)
        g = np.arange(NG)
        gridnp[:, 0:NG] = lin[(g % 2)[None, :] * 128 + np.arange(C)[:, None]] - 1.0
        gridnp[:, NG:2 * NG] = (lin[r0 + g // 2] - 1.0)[None, :]
        in_maps.append({
            "XS": xs_np.reshape(C, XSIZE), "WT": WTnp, "WH": WHnp,
            "GRID": gridnp,
        })
    return in_maps


def _refine_scores_f64(x_img, Wc64, Wlog64, pix):
    r, c = pix // W, pix % W
    xpad = np.pad(x_img.astype(np.float64), ((0, 0), (1, 1), (1, 1)))
    pat = np.empty((len(pix), x_img.shape[0], 3, 3))
    for kh in range(3):
        for kw in range(3):
            pat[:, :, kh, kw] = xpad[:, r + kh, c + kw].T
    t = np.maximum(np.einsum("pikl,oikl->po", pat, Wc64, optimize=True), 0.0)
    return t @ Wlog64


def kernel(**inputs):
    in_maps = _prepare_in_maps(inputs)
    from concourse.bass_utils import run_bass_kernel_spmd
    global LAST_RESULT
    res = run_bass_kernel_spmd(_program(), in_maps, list(range(8)), trace=TRACE)
    LAST_RESULT = res
    return _assemble([res.results[c] for c in range(8)], inputs)


def _assemble(results, inputs):
    out = np.empty((NIMG, H * W, 5), np.float32)
    for core in range(8):
        img, s = core // 4, core % 4
        out[img, s * NPIX:(s + 1) * NPIX, :] = results[core]["OUT"]

    x = np.asarray(inputs["x"], dtype=np.float32)
    Wc64 = np.asarray(inputs["W_conv"], dtype=np.float64)
    Wlog64 = np.asarray(inputs["W_log"], dtype=np.float64)[0]

    kept_boxes = np.empty((NIMG, 256, 4), np.float32)
    kept_scores = np.empty((NIMG, 256), np.float32)
    for img in range(NIMG):
        sc = out[img, :, 0]
        cand = np.argpartition(-sc, 2048)[:2048]
        # refine candidate scores in f64: reference-fp32 ordering equals
        # f64 ordering empirically, and the device/reference fp32 errors
        # both sit near the smallest top-k score gaps.
        sc64 = _refine_scores_f64(x[img], Wc64, Wlog64, cand)
        order = np.lexsort((cand, -sc64))
        pre = cand[order[:1000]]
        sup = _greedy_nms_fallback(sc[pre], out[img, pre, 1:5])
        keep_sc = np.where(sup, -np.inf, sc[pre])
        keep_sc64 = np.where(sup, -np.inf, sc64[order[:1000]])
        forder = np.lexsort((np.arange(len(pre)), -keep_sc64))[:256]
        kept_scores[img] = keep_sc[forder]
        kept_boxes[img] = out[img, pre[forder], 1:5]
    return out, kept_boxes, kept_scores
